# revision 1
# baseline (speedup 1.0000x reference)
"""Trainium2 Bass kernel for MessageControlGraphAttentionLayer.

Shapes (hardcoded): x (4,256,256) f32, boundary (4,256) int32,
att_proj_w (256,256), att_proj_b (256,), att_weight (256,8),
proj_att_w (2048,256), proj_att_b (256,), proj_no_w (256,256),
proj_no_b (256,), bn_gamma (256,), bn_beta (256,).

Sharding: 8 cores, core c handles batch b=c//2, query rows
j in [128*(c%2), 128*(c%2)+128). All weights replicated. BN batch
stats are all-reduced across the 8 cores with a device collective.

Math (per core, J=128 query rows, T=256 keys, D=O=256, H=8):
  mm1: logits_j[o,k] = sum_d W1[d,o] * (x[b,k,d]*x[b,j,d])   (PE, fp32)
       rhs_j = xT * xT[:,j] per-partition scale (DVE/GPSIMD)
  tanh(+b1) on ACT, psum->sbuf
  mm2: att[(j,h),k] += W2sp[j-slot].T @ a_j  -- W2 embedded in zero-padded
       (128,128) tiles so 16 j's * 8 heads pack densely into 128 psum
       partitions per block.
  mask-mul + exp (accum row sums) + 1/Z scale -> normalized attention
  PE-transpose (jh,k)->(k,jh); mm3: x1T[d,(j,h)] = xk.T @ enT
  mm4: y[o,j] = sum_h Wph[h].T @ x1T[:,:,h] + Wn.T @ xT[:,my j] (+biases)
  BN stats (sum, sumsq) -> AllReduce over 8 cores -> affine + selu.
"""

import sys

if "/opt/trn_rl_repo" not in sys.path:
    sys.path.insert(0, "/opt/trn_rl_repo")

import numpy as np

B, T, D, O, H = 4, 256, 256, 256, 8
P = 128
NCORES = 8
J = 128  # query rows per core
NBLK = 8  # blocks of 16 j per core
BN_EPS = 1e-5
SELU_LAM = 1.0507009873554805
SELU_ALPHA = 1.6732632423543772

_CACHE = {}
_CACHE_ETP = [None]


def _message_control_mask_np(boundary):
    Bb, Tt = boundary.shape
    s = np.cumsum(boundary.astype(np.int64), axis=1)
    spad = np.concatenate([np.zeros((Bb, 1), np.int64), s], axis=1)  # (B,T+1)
    idx = np.arange(Tt)
    jj, kk = np.meshgrid(idx, idx, indexing="ij")
    hi = np.maximum(jj, kk)
    lo = np.minimum(jj, kk)
    rng_sum = spad[:, hi + 1] - spad[:, lo]  # (B,T,T)
    mask = rng_sum == 0
    mask = mask | np.eye(Tt, dtype=bool)[None]
    return mask.astype(np.float32)


def _build_module(with_collective=True, reps=1):
    from concourse import bacc, bass, tile
    import concourse.mybir as mybir
    from concourse.masks import make_identity

    f32 = mybir.dt.float32
    f32r = mybir.dt.float32r  # single-pass fp32 matmul: 4x faster, ~1e-4 rel
    AF = mybir.ActivationFunctionType
    ALU = mybir.AluOpType

    nc = bacc.Bacc("TRN2", target_bir_lowering=False, debug=False,
                   num_devices=NCORES)

    xT_d = nc.dram_tensor("xT", [D, T], f32, kind="ExternalInput")
    xk_d = nc.dram_tensor("xk", [T, D], f32r, kind="ExternalInput")
    w1_d = nc.dram_tensor("w1", [D, O], f32r, kind="ExternalInput")
    w2_d = nc.dram_tensor("w2", [O, H], f32, kind="ExternalInput")
    wph_d = nc.dram_tensor("wph", [H, 2, P, O], f32, kind="ExternalInput")
    wn_d = nc.dram_tensor("wn", [D, O], f32, kind="ExternalInput")
    maskx_d = nc.dram_tensor("maskx", [P, NBLK, T], f32, kind="ExternalInput")
    pvec_d = nc.dram_tensor("pvec", [P, 8], f32, kind="ExternalInput")
    yout_d = nc.dram_tensor("yout", [2, P, J], f32, kind="ExternalOutput")

    with tile.TileContext(nc) as tc:
        with (
            tc.tile_pool(name="const", bufs=1) as cpool,
            tc.tile_pool(name="dram", bufs=1, space="DRAM") as dpool,
        ):
            # Tiny dummy Tanh first: forces the ACT table load (a TDRAM DMA)
            # to be queued before the multi-MB const loads, so the first real
            # tanh isn't gated ~10us on DMA traffic.
            warm = cpool.tile([P, 1], f32)
            nc.gpsimd.memset(warm[:], 0.0)
            nc.scalar.activation(warm[:], warm[:],
                                 mybir.ActivationFunctionType.Tanh)
            pvec_sb = cpool.tile([P, 8], f32)
            nc.sync.dma_start(pvec_sb[:], pvec_d[:])
            xT_sb = cpool.tile([P, 2, T], f32)
            xT_r = xT_d.ap().rearrange("(c p) k -> p c k", p=P)
            nc.sync.dma_start(xT_sb[:, 0, :], xT_r[:, 0, :])
            nc.sync.dma_start(xT_sb[:, 1, :], xT_r[:, 1, :])
            w1_sb = cpool.tile([P, 2, O], f32r)
            nc.sync.dma_start(w1_sb[:], w1_d.ap().rearrange("(c p) o -> p c o", p=P))
            xk_sb = cpool.tile([P, 2, D], f32r)
            nc.sync.dma_start(xk_sb[:], xk_d.ap().rearrange("(c p) d -> p c d", p=P))
            # Build the 32 zero-padded mm2 weight tiles on-device from the
            # tiny (O,H) att_weight: tile (jl,oc) holds W2[oc-chunk] at
            # columns [8*jl, 8*jl+8) so 16 j's pack densely into 128 psum
            # partitions per block.
            w2_sb = cpool.tile([P, 2, H], f32)
            nc.sync.dma_start(w2_sb[:], w2_d.ap().rearrange("(c p) h -> p c h", p=P))
            w2sp_sb = cpool.tile([P, 32, P], f32r)
            zf = cpool.tile([P, P], f32)
            nc.gpsimd.memset(zf[:], 0.0)
            for t_ in range(32):
                nc.vector.tensor_copy(w2sp_sb[:, t_, :], zf[:])
            for jl in range(16):
                for oc in range(2):
                    nc.vector.tensor_copy(
                        w2sp_sb[:, jl * 2 + oc, 8 * jl:8 * jl + 8],
                        w2_sb[:, oc, :])
            maskx_sb = cpool.tile([P, NBLK, T], f32)
            nc.sync.dma_start(maskx_sb[:], maskx_d[:])
            wn_sb = cpool.tile([P, 2, O], f32)
            nc.sync.dma_start(wn_sb[:], wn_d.ap().rearrange("(c p) o -> p c o", p=P))
            # wph is only needed by phase 3 -- load it last
            wph_sb = cpool.tile([P, 16, O], f32)
            nc.sync.dma_start(wph_sb[:], wph_d.ap().rearrange("h c p o -> p (h c) o"))
            ident = cpool.tile([P, P], f32)
            make_identity(nc, ident[:])
            identr = cpool.tile([P, P], f32r)
            nc.vector.tensor_copy(identr[:], ident[:])
            x1T_a = cpool.tile([P, J, H], f32)
            x1T_b = cpool.tile([P, J, H], f32)
            x1T = [x1T_a, x1T_b]

            with (
                tc.tile_pool(name="work", bufs=1) as wpool,
                tc.tile_pool(name="pp1", bufs=4, space="PSUM") as pp1,
                tc.tile_pool(name="pp4", bufs=1, space="PSUM") as pp4,
                tc.tile_pool(name="pp2", bufs=1, space="PSUM") as pp2,
                tc.tile_pool(name="ppx", bufs=2, space="PSUM") as ppx,
            ):
                # Host rolls the key axis by -j0 per core, so each core's
                # query columns are always 0..127 of xT (SPMD: one program).
                for _rep in range(reps):
                    for blk in range(NBLK):
                        psum2 = pp2.tile([P, T], f32, tag="p2", name=f"p2_{blk}")
                        for gg in range(8):
                            g = blk * 8 + gg
                            ps1 = [
                                pp1.tile([P, 2, T], f32, tag="p1", name=f"p1a_{g}"),
                                pp1.tile([P, 2, T], f32, tag="p1", name=f"p1b_{g}"),
                            ]
                            a_t = wpool.tile([P, 2, 2, T], f32r, tag="a", bufs=6,
                                             name=f"a_{g}")
                            rhs = {}
                            for jj in range(2):
                                jl = g * 2 + jj  # local query index 0..127
                                for dc in range(2):
                                    r = wpool.tile([P, T], f32r, tag="rhs", bufs=16,
                                                   name=f"rhs_{g}_{jj}_{dc}")
                                    use_pool = (dc == 1) and (g % 2 == 0)
                                    eng = nc.gpsimd if use_pool else nc.vector
                                    eng.tensor_scalar_mul(
                                        out=r[:],
                                        in0=xT_sb[:, dc, :],
                                        scalar1=xT_sb[:, dc, jl:jl + 1],
                                    )
                                    rhs[(jj, dc)] = r
                            for oc in range(2):
                                for jj in range(2):
                                    for dc in range(2):
                                        nc.tensor.matmul(
                                            ps1[oc][:, jj, :],
                                            w1_sb[:, dc, oc * P:(oc + 1) * P],
                                            rhs[(jj, dc)][:],
                                            start=(dc == 0),
                                            stop=(dc == 1),
                                        )
                            for oc in range(2):
                                nc.scalar.activation(
                                    a_t[:, oc, :, :], ps1[oc][:],
                                    AF.Tanh, bias=pvec_sb[:, oc:oc + 1],
                                )
                            for jj in range(2):
                                jl_blk = gg * 2 + jj  # 0..15 within block
                                for oc in range(2):
                                    nc.tensor.matmul(
                                        psum2[:],
                                        w2sp_sb[:, jl_blk * 2 + oc, :],
                                        a_t[:, oc, jj, :],
                                        start=(gg == 0 and jj == 0 and oc == 0),
                                        stop=(gg == 7 and jj == 1 and oc == 1),
                                    )
                        # --- block tail: mask, exp, normalize, transpose, mm3 ---
                        attm = wpool.tile([P, T], f32, tag="attm", bufs=3,
                                          name=f"attm_{blk}")
                        nc.vector.tensor_mul(attm[:], psum2[:], maskx_sb[:, blk, :])
                        e_t = wpool.tile([P, T], f32, tag="e", bufs=3,
                                         name=f"e_{blk}")
                        zsum = wpool.tile([P, 1], f32, tag="zs", bufs=2,
                                          name=f"zs_{blk}")
                        nc.scalar.activation(e_t[:], attm[:], AF.Exp,
                                             accum_out=zsum[:])
                        zinv = wpool.tile([P, 1], f32, tag="zi", bufs=2,
                                          name=f"zi_{blk}")
                        nc.vector.reciprocal(zinv[:], zsum[:])
                        en = wpool.tile([P, T], f32r, tag="en", bufs=3,
                                        name=f"en_{blk}")
                        nc.vector.tensor_scalar_mul(out=en[:], in0=e_t[:],
                                                    scalar1=zinv[:])
                        if blk % 2 == 0:
                            eTp = [
                                wpool.tile([P, 2, P], f32r, tag=f"eT{kc}", bufs=3,
                                           name=f"eT_{blk}_{kc}")
                                for kc in range(2)
                            ]
                            _CACHE_ETP[0] = eTp
                        else:
                            eTp = _CACHE_ETP[0]
                        for kc in range(2):
                            psT = ppx.tile([P, P], f32r, tag="px",
                                           name=f"psT_{blk}_{kc}")
                            nc.tensor.transpose(psT[:], en[:, kc * P:(kc + 1) * P],
                                                identr[:])
                            nc.vector.tensor_copy(eTp[kc][:, blk % 2, :], psT[:])
                        if blk % 2 == 1:
                            pair = blk // 2
                            for md in range(2):
                                ps3 = ppx.tile([P, 2, P], f32, tag="px",
                                               name=f"ps3_{blk}_{md}")
                                for kc in range(2):
                                    nc.tensor.matmul(
                                        ps3[:],
                                        xk_sb[:, kc, md * P:(md + 1) * P],
                                        eTp[kc][:],
                                        start=(kc == 0),
                                        stop=(kc == 1),
                                    )
                                nc.vector.tensor_copy(
                                    x1T[md][:, pair * 32:(pair + 1) * 32, :],
                                    ps3[:].rearrange("p a (b c) -> p (a b) c", c=H),
                                )

                    # ---------------- phase 3: output projections ----------------
                    y_t = []
                    stats = wpool.tile([P, 4], f32, tag="stats", name="stats")
                    for oc in range(2):
                        ps4 = pp4.tile([P, J], f32, tag="p4", name=f"ps4_{oc}")
                        # split over j-halves: half 0 only needs blocks 0-3,
                        # so its matmuls can fill PE idle while blocks 4-7
                        # are still in flight.
                        for jh in range(2):
                            js = slice(jh * 64, (jh + 1) * 64)
                            first = True
                            for h in range(H):
                                for dc in range(2):
                                    nc.tensor.matmul(
                                        ps4[:, js],
                                        wph_sb[:, h * 2 + dc,
                                               oc * P:(oc + 1) * P],
                                        x1T[dc][:, js, h],
                                        start=first, stop=False,
                                    )
                                    first = False
                            for dc in range(2):
                                nc.tensor.matmul(
                                    ps4[:, js],
                                    wn_sb[:, dc, oc * P:(oc + 1) * P],
                                    xT_sb[:, dc, js],
                                    start=False, stop=(dc == 1),
                                )
                        yt = wpool.tile([P, J], f32, tag=f"y{oc}", name=f"y_{oc}")
                        nc.scalar.activation(yt[:], ps4[:], AF.Identity,
                                             bias=pvec_sb[:, 2 + oc:3 + oc],
                                             accum_out=stats[:, oc:oc + 1])
                        y_t.append(yt)
                        sq = wpool.tile([P, J], f32, tag="sq", bufs=2,
                                        name=f"sq_{oc}")
                        nc.scalar.activation(sq[:], yt[:], AF.Square,
                                             accum_out=stats[:, 2 + oc:3 + oc])

                    # ---------------- BN all-reduce + affine + selu ----------------
                    cc_in = dpool.tile([P, 4], f32, name="cc_in")
                    cc_out = dpool.tile([P, 4], f32, addr_space="Shared",
                                        name="cc_out")
                    nc.sync.dma_start(cc_in[:], stats[:])
                    if with_collective:
                        nc.gpsimd.collective_compute(
                            "AllReduce",
                            ALU.add,
                            replica_groups=[list(range(NCORES))],
                            ins=[cc_in.opt()],
                            outs=[cc_out.opt()],
                        )
                    else:  # perf-model probe only: skip the collective
                        nc.sync.dma_start(cc_out[:], cc_in[:])
                    statg = wpool.tile([P, 4], f32, tag="statg", name="statg")
                    nc.sync.dma_start(statg[:], cc_out[:])

                    NTOT = float(B * T)

                    def wt2(nm):
                        return wpool.tile([P, 2], f32, tag=nm, name=nm)

                    # statg cols: [s1_oc0, s1_oc1, s2_oc0, s2_oc1]
                    mom = wpool.tile([P, 4], f32, tag="mom", name="mom")
                    nc.vector.tensor_scalar_mul(out=mom[:, 0:2],
                                                in0=statg[:, 0:2],
                                                scalar1=1.0 / NTOT)
                    nc.vector.tensor_scalar(out=mom[:, 2:4],
                                            in0=statg[:, 2:4],
                                            scalar1=1.0 / NTOT,
                                            scalar2=BN_EPS,
                                            op0=ALU.mult, op1=ALU.add)
                    mu = mom[:, 0:2]
                    varp = mom[:, 2:4]
                    musq = wt2("musq")
                    nc.vector.tensor_mul(musq[:], mu, mu)
                    nc.vector.tensor_sub(varp, varp, musq[:])
                    # rsqrt on DVE only (no ACT table swap): quake guess + 3
                    # Newton iterations -> ~1ulp fp32.
                    i32 = mybir.dt.int32
                    magic = wpool.tile([P, 2], i32, tag="magic", name="magic")
                    nc.vector.memset(magic[:], 0x5F3759DF)
                    ri = wpool.tile([P, 2], i32, tag="ri", name="ri")
                    nc.vector.tensor_scalar(out=ri[:], in0=varp.bitcast(i32),
                                            scalar1=1, scalar2=None,
                                            op0=ALU.arith_shift_right)
                    nc.vector.tensor_sub(ri[:], magic[:], ri[:])
                    rstd = wt2("rstd")
                    nc.vector.tensor_copy(rstd[:], ri[:].bitcast(f32))
                    ra = wt2("ra")
                    rb = wt2("rb")
                    for _ in range(2):
                        nc.vector.tensor_mul(ra[:], rstd[:], rstd[:])
                        nc.vector.scalar_tensor_tensor(
                            out=rb[:], in0=ra[:], scalar=-0.5, in1=varp,
                            op0=ALU.mult, op1=ALU.mult)
                        nc.vector.tensor_scalar_add(out=rb[:], in0=rb[:],
                                                    scalar1=1.5)
                        nc.vector.tensor_mul(rstd[:], rstd[:], rb[:])
                    scl = wt2("scl")
                    nc.vector.tensor_mul(scl[:], pvec_sb[:, 4:6], rstd[:])
                    tmp = wt2("tmp")
                    nc.vector.tensor_mul(tmp[:], mu, scl[:])
                    shf = wt2("shf")
                    nc.vector.tensor_sub(shf[:], pvec_sb[:, 6:8], tmp[:])

                    z = wpool.tile([P, 2, J], f32, tag="z", name="z")
                    for oc in range(2):
                        nc.vector.tensor_scalar(out=z[:, oc, :], in0=y_t[oc][:],
                                                scalar1=scl[:, oc:oc + 1],
                                                scalar2=shf[:, oc:oc + 1],
                                                op0=ALU.mult, op1=ALU.add)
                    # selu on the merged (P, 2*J) tile
                    neg = wpool.tile([P, 2, J], f32, tag="neg", name="neg")
                    nc.vector.tensor_scalar_min(out=neg[:], in0=z[:], scalar1=0.0)
                    ep = wpool.tile([P, 2, J], f32, tag="ep", name="ep")
                    nc.scalar.activation(ep[:], neg[:], AF.Exp)
                    em = wpool.tile([P, 2, J], f32, tag="em", name="em")
                    nc.vector.tensor_scalar(
                        out=em[:], in0=ep[:],
                        scalar1=SELU_LAM * SELU_ALPHA,
                        scalar2=-SELU_LAM * SELU_ALPHA,
                        op0=ALU.mult, op1=ALU.add)
                    pos = wpool.tile([P, 2, J], f32, tag="pos", name="pos")
                    nc.vector.tensor_scalar_max(out=pos[:], in0=z[:], scalar1=0.0)
                    outz = wpool.tile([P, 2, J], f32, tag="outz", name="outz")
                    nc.vector.scalar_tensor_tensor(
                        out=outz[:], in0=pos[:], scalar=SELU_LAM, in1=em[:],
                        op0=ALU.mult, op1=ALU.add)
                    nc.sync.dma_start(yout_d.ap().rearrange("c p j -> p c j"),
                                      outz[:])

    nc.compile()
    return nc


def _prep_inputs(x, boundary, att_proj_w, att_proj_b, att_weight,
                 proj_att_w, proj_att_b, proj_no_w, proj_no_b,
                 bn_gamma, bn_beta):
    mask = _message_control_mask_np(np.asarray(boundary))
    x = np.ascontiguousarray(np.asarray(x, dtype=np.float32))
    w1 = np.ascontiguousarray(np.asarray(att_proj_w, dtype=np.float32))
    w2 = np.ascontiguousarray(np.asarray(att_weight, dtype=np.float32))
    wph = np.ascontiguousarray(
        np.asarray(proj_att_w, dtype=np.float32)
        .reshape(D, H, O).transpose(1, 0, 2).reshape(H, 2, P, O))
    wn = np.ascontiguousarray(np.asarray(proj_no_w, dtype=np.float32))

    by = (np.asarray(proj_att_b, dtype=np.float32)
          + np.asarray(proj_no_b, dtype=np.float32))
    pvec = np.zeros((P, 8), dtype=np.float32)
    b1 = np.asarray(att_proj_b, dtype=np.float32)
    g = np.asarray(bn_gamma, dtype=np.float32)
    be = np.asarray(bn_beta, dtype=np.float32)
    for oc in range(2):
        pvec[:, oc] = b1[oc * P:(oc + 1) * P]
        pvec[:, 2 + oc] = by[oc * P:(oc + 1) * P]
        pvec[:, 4 + oc] = g[oc * P:(oc + 1) * P]
        pvec[:, 6 + oc] = be[oc * P:(oc + 1) * P]

    in_maps = []
    for c in range(NCORES):
        b = c // 2
        j0 = (c % 2) * J
        xb = x[b]  # (T, D)
        xT = np.ascontiguousarray(xb.T)  # (D, T)
        # roll keys so this core's query columns are always 0..127
        xTq = np.ascontiguousarray(np.roll(xT, -j0, axis=1))
        xkq = np.ascontiguousarray(np.roll(xb, -j0, axis=0))
        m = mask[b, j0:j0 + J]  # (J, T) in original key order
        mq = np.roll(m, -j0, axis=1)
        maskx = np.ascontiguousarray(
            np.repeat(mq.reshape(NBLK, 16, 1, T), H, axis=2)
            .transpose(1, 2, 0, 3).reshape(P, NBLK, T))
        in_maps.append({
            "xT": xTq,
            "xk": xkq,
            "w1": w1,
            "w2": w2,
            "wph": wph,
            "wn": wn,
            "maskx": maskx,
            "pvec": pvec,
        })
    return in_maps


def kernel(**inputs):
    from concourse.bass_utils import run_bass_kernel_spmd

    if "nc" not in _CACHE:
        _CACHE["nc"] = _build_module()
    nc = _CACHE["nc"]

    in_maps = _prep_inputs(**inputs)
    res = run_bass_kernel_spmd(nc, in_maps, core_ids=list(range(NCORES)),
                               **_CACHE.get("run_kwargs", {}))
    _CACHE["last_results"] = res

    out = np.zeros((B, T, O), dtype=np.float32)
    for c in range(NCORES):
        b = c // 2
        j0 = (c % 2) * J
        yc = res.results[c]["yout"]  # (2, P, J): (oc, o_sub, j_local)
        # keys were rolled but output rows are the queries (j local order is
        # 0..127 == global j0..j0+127); columns are o (unrolled). The roll
        # only permuted the key/contraction axis, which is summed out.
        out[b, j0:j0 + J, :] = yc.reshape(O, J).T
    return out


if __name__ == "__main__":
    # smoke build
    _build_module()
    print("build ok")



# revision 10
# speedup vs baseline: 3.0761x; 3.0761x over previous
"""Trainium2 Bass kernel for MessageControlGraphAttentionLayer (sparse form).

Key insight: mask[j,k]=1 only when j,k lie in the same zero-run of
boundary (plus the diagonal), so only ~0.8% of (j,k) pairs carry a
computed logit. For masked pairs exp(0)=1, so softmax collapses:
  Z[j,h]   = (T - L_j) + sum_{k in seg(j)} exp(l[j,k,h])
  x1[j,:,h]= ( (A - S_seg(j)) + sum_{k in seg(j)} exp(l)*x[k] ) / Z
with A = sum_k x[k,:] and S_seg = sum_{k in seg} x[k,:]. A - S comes
from one matmul with (1 - mask); the kept pairs live on a compact
pair axis p (host-gathered columns XJg/XKg, selector matrices JSEL).

Per core (batch b=c//2, rows j in [128*(c%2), ...+128)):
  PP[d,p] = XJg*XKg (DVE) -> mm1 (4 matmuls, bf16) -> tanh+b1 (ACT)
  mm2: l[p,h] via lhsT=a-chunk, rhs=W2 (N=8) -> exp -> Z via JSEL
  zi = 1/((T-L)+Z) -> zin gather via JSELT -> en = exp*zin
  JSELh[p,j] = en[:,h]*JSEL (per h); x1T[d,j] += Xp^T @ JSELh
  pseudo-chunk: (A-S)^T with diag(zi_h) rhs handles the masked mass.
  mm4: y = Wp^T x1 + Wn^T x + biases; BN stats AllReduce; affine+selu.

All weights/x in bf16 (validated ~4e-3 end-to-end rel err).
"""

import sys

if "/opt/trn_rl_repo" not in sys.path:
    sys.path.insert(0, "/opt/trn_rl_repo")

import numpy as np
import ml_dtypes

B, T, D, O, H = 4, 256, 256, 256, 8
P = 128
NCORES = 8
J = 128  # rows per core
BN_EPS = 1e-5
SELU_LAM = 1.0507009873554805
SELU_ALPHA = 1.6732632423543772

BF = ml_dtypes.bfloat16

_CACHE = {}


def _message_control_mask_np(boundary):
    Bb, Tt = boundary.shape
    s = np.cumsum(boundary.astype(np.int64), axis=1)
    spad = np.concatenate([np.zeros((Bb, 1), np.int64), s], axis=1)
    idx = np.arange(Tt)
    jj, kk = np.meshgrid(idx, idx, indexing="ij")
    rng_sum = spad[:, np.maximum(jj, kk) + 1] - spad[:, np.minimum(jj, kk)]
    mask = rng_sum == 0
    mask = mask | np.eye(Tt, dtype=bool)[None]
    return mask.astype(np.float32)


def _seg_of(brow):
    """Per-row (k0, L): the maximal zero-run containing the row, or the
    singleton (j, 1) for boundary rows (diagonal-only)."""
    seg = np.zeros((T, 2), np.int64)
    i = 0
    while i < T:
        if brow[i] == 0:
            j = i
            while j < T and brow[j] == 0:
                j += 1
            seg[i:j, 0] = i
            seg[i:j, 1] = j - i
            i = j
        else:
            seg[i] = (i, 1)
            i += 1
    return seg


def _build_module(with_collective=True, reps=1, nch=None):
    from concourse import bacc, tile
    import concourse.mybir as mybir
    from concourse.masks import make_identity

    if nch is None:
        nch = _CACHE.get("nch", 3)
    NCH = nch
    PAD = NCH * P

    f32 = mybir.dt.float32
    bf16 = mybir.dt.bfloat16
    AF = mybir.ActivationFunctionType
    ALU = mybir.AluOpType

    nc = bacc.Bacc("TRN2", target_bir_lowering=False, debug=False,
                   num_devices=NCORES)

    xjg_d = nc.dram_tensor("xjg", [P, 2, PAD], bf16, kind="ExternalInput")
    xkg_d = nc.dram_tensor("xkg", [P, 2, PAD], bf16, kind="ExternalInput")
    xp_d = nc.dram_tensor("xp", [P, NCH, 2, P], bf16, kind="ExternalInput")
    jsel_d = nc.dram_tensor("jsel", [P, NCH, P], bf16, kind="ExternalInput")
    jselt_d = nc.dram_tensor("jselt", [P, NCH, P], bf16, kind="ExternalInput")
    notm_d = nc.dram_tensor("notm", [P, 2, P], bf16, kind="ExternalInput")
    xk_d = nc.dram_tensor("xk", [P, 2, 2, P], bf16, kind="ExternalInput")
    xtl_d = nc.dram_tensor("xtl", [P, 2, P], bf16, kind="ExternalInput")
    w1_d = nc.dram_tensor("w1l", [P, 2, 2, P], bf16, kind="ExternalInput")
    w2_d = nc.dram_tensor("w2c", [P, 2, H], bf16, kind="ExternalInput")
    wpl_d = nc.dram_tensor("wpl", [P, 16, 2, P], bf16, kind="ExternalInput")
    wnl_d = nc.dram_tensor("wnl", [P, 2, 2, P], bf16, kind="ExternalInput")
    zc_d = nc.dram_tensor("zc", [P, 1], f32, kind="ExternalInput")
    pvec_d = nc.dram_tensor("pvec", [P, 8], f32, kind="ExternalInput")
    yout_d = nc.dram_tensor("yout", [2, P, J], f32, kind="ExternalOutput")

    with tile.TileContext(nc) as tc:
        with (
            tc.tile_pool(name="const", bufs=1) as cpool,
            tc.tile_pool(name="dram", bufs=1, space="DRAM") as dpool,
        ):
            # Force the ACT table (Tanh/Exp/Identity/Square set) to load
            # before the big const DMAs queue up.
            warm = cpool.tile([P, 1], f32)
            nc.gpsimd.memset(warm[:], 0.0)
            nc.scalar.activation(warm[:], warm[:], AF.Tanh)

            pvec_sb = cpool.tile([P, 8], f32)
            nc.sync.dma_start(pvec_sb[:], pvec_d[:])
            zc_sb = cpool.tile([P, 1], f32)
            nc.sync.dma_start(zc_sb[:], zc_d[:])
            xjg = cpool.tile([P, 2, PAD], bf16)
            nc.sync.dma_start(xjg[:], xjg_d[:])
            xkg = cpool.tile([P, 2, PAD], bf16)
            nc.sync.dma_start(xkg[:], xkg_d[:])
            w1_sb = cpool.tile([P, 2, 2, P], bf16)
            nc.sync.dma_start(w1_sb[:], w1_d[:])
            w2_sb = cpool.tile([P, 2, H], bf16)
            nc.sync.dma_start(w2_sb[:], w2_d[:])
            jsel = cpool.tile([P, NCH, P], bf16)
            nc.sync.dma_start(jsel[:], jsel_d[:])
            jselt = cpool.tile([P, NCH, P], bf16)
            nc.sync.dma_start(jselt[:], jselt_d[:])
            notm = cpool.tile([P, 2, P], bf16)
            nc.sync.dma_start(notm[:], notm_d[:])
            xk_sb = cpool.tile([P, 2, 2, P], bf16)
            nc.sync.dma_start(xk_sb[:], xk_d[:])
            xp_sb = cpool.tile([P, NCH, 2, P], bf16)
            nc.sync.dma_start(xp_sb[:], xp_d[:])
            xtl = cpool.tile([P, 2, P], bf16)
            nc.sync.dma_start(xtl[:], xtl_d[:])
            wnl = cpool.tile([P, 2, 2, P], bf16)
            nc.sync.dma_start(wnl[:], wnl_d[:])
            wpl = cpool.tile([P, 16, 2, P], bf16)
            nc.sync.dma_start(wpl[:], wpl_d[:])
            identf = cpool.tile([P, P], f32)
            make_identity(nc, identf[:])
            identr = cpool.tile([P, P], bf16)
            nc.vector.tensor_copy(identr[:], identf[:])

            with (
                tc.tile_pool(name="work", bufs=1) as wpool,
                tc.tile_pool(name="pp1", bufs=1, space="PSUM") as pp1,
                tc.tile_pool(name="pps", bufs=1, space="PSUM") as pps,
                tc.tile_pool(name="ppx", bufs=1, space="PSUM") as ppx,
                tc.tile_pool(name="pp4", bufs=1, space="PSUM") as pp4,
            ):
                for _rep in range(reps):
                    # ---- phase 1: pair products -> mm1 -> tanh ----
                    ppt = wpool.tile([P, 2, PAD], bf16, tag="pp", name="ppt")
                    for dc in range(2):
                        nc.vector.tensor_mul(ppt[:, dc, :], xjg[:, dc, :],
                                             xkg[:, dc, :])
                    ps1 = [pp1.tile([P, PAD], f32, tag=f"p1_{oc}",
                                    name=f"ps1_{oc}") for oc in range(2)]
                    for oc in range(2):
                        for dc in range(2):
                            nc.tensor.matmul(ps1[oc][:],
                                             w1_sb[:, dc, oc, :],
                                             ppt[:, dc, :],
                                             start=(dc == 0), stop=(dc == 1))
                    a_t = wpool.tile([P, 2, PAD], bf16, tag="a", name="a_t")
                    for oc in range(2):
                        nc.scalar.activation(a_t[:, oc, :], ps1[oc][:],
                                             AF.Tanh,
                                             bias=pvec_sb[:, oc:oc + 1])

                    # ---- phase 2: mm2 -> exp per chunk; Z accumulation ----
                    # one shared 2KB psum bank for all the tiny [P, 8] psums:
                    # [0:8]=psz, [8+8ch]=pszn, [8+8(NCH+ch)]=psl, [256:512]=AS
                    small = pps.tile([P, 512], f32, tag="small", name="small")
                    psz = small[:, 0:H]
                    expp = wpool.tile([P, NCH, H], bf16, tag="expp",
                                      name="expp")
                    for ch in range(NCH):
                        psl = small[:, H + H * (NCH + ch):
                                    H + H * (NCH + ch) + H]
                        for oc in range(2):
                            nc.tensor.matmul(psl,
                                             a_t[:, oc, ch * P:(ch + 1) * P],
                                             w2_sb[:, oc, :],
                                             start=(oc == 0), stop=(oc == 1))
                        nc.scalar.activation(expp[:, ch, :], psl, AF.Exp)
                    for ch in range(NCH):
                        nc.tensor.matmul(psz, jsel[:, ch, :],
                                         expp[:, ch, :],
                                         start=(ch == 0), stop=(ch == NCH - 1))
                    zs = wpool.tile([P, H], f32, tag="zs", name="zs")
                    nc.vector.tensor_scalar_add(out=zs[:], in0=psz,
                                                scalar1=zc_sb[:])
                    zi = wpool.tile([P, H], f32, tag="zi", name="zi")
                    nc.vector.reciprocal(zi[:], zs[:])
                    zib = wpool.tile([P, H], bf16, tag="zib", name="zib")
                    nc.vector.tensor_copy(zib[:], zi[:])

                    # ---- A - S side chain (overlaps with above) ----
                    psas = small[:, 256:512].rearrange("p (a b) -> p a b", a=2)
                    for md in range(2):
                        for kc in range(2):
                            nc.tensor.matmul(psas[:, md, :],
                                             xk_sb[:, kc, md, :],
                                             notm[:, kc, :],
                                             start=(kc == 0), stop=(kc == 1))
                    assb = wpool.tile([P, 2, P], bf16, tag="assb", name="assb")
                    nc.scalar.activation(assb[:], psas, AF.Identity)
                    ast = wpool.tile([P, 2, P], bf16, tag="ast", name="ast")
                    pstt = pps.tile([P, 2, P], bf16, tag="pst", name="pstt")
                    for md in range(2):
                        nc.tensor.transpose(pstt[:, md, :], assb[:, md, :],
                                            identr[:])
                        nc.vector.tensor_copy(ast[:, md, :], pstt[:, md, :])

                    # ---- phase 3: zin gather, en, JSELh builds ----
                    jh = wpool.tile([P, NCH, H, P], bf16, tag="jh", name="jh")
                    for ch in range(NCH):
                        pszn = small[:, H + H * ch:H + H * ch + H]
                        nc.tensor.matmul(pszn, jselt[:, ch, :], zib[:],
                                         start=True, stop=True)
                        en = wpool.tile([P, H], f32, tag="en", bufs=3,
                                        name=f"en_{ch}")
                        nc.vector.tensor_mul(en[:], expp[:, ch, :], pszn)
                        for h in range(H):
                            eng = [nc.vector, nc.gpsimd][h % 2]
                            eng.tensor_scalar_mul(out=jh[:, ch, h, :],
                                                  in0=jsel[:, ch, :],
                                                  scalar1=en[:, h:h + 1])
                    jhps = wpool.tile([P, H, P], bf16, tag="jhps", name="jhps")
                    for h in range(H):
                        eng = [nc.vector, nc.gpsimd][h % 2]
                        eng.tensor_scalar_mul(out=jhps[:, h, :],
                                              in0=identr[:],
                                              scalar1=zi[:, h:h + 1])

                    # ---- phase 4: x1T accumulation (2 md waves) ----
                    x1sb = wpool.tile([P, 2, H, P], bf16, tag="x1sb",
                                      name="x1sb")
                    for md in range(2):
                        psx1 = ppx.tile([P, H, P], f32, tag="psx1",
                                        name=f"psx1_{md}")
                        for h in range(H):
                            for ch in range(NCH):
                                nc.tensor.matmul(psx1[:, h, :],
                                                 xp_sb[:, ch, md, :],
                                                 jh[:, ch, h, :],
                                                 start=(ch == 0), stop=False)
                            nc.tensor.matmul(psx1[:, h, :], ast[:, md, :],
                                             jhps[:, h, :],
                                             start=False, stop=True)
                        if md == 0:
                            nc.scalar.activation(x1sb[:, md, :, :], psx1[:],
                                                 AF.Identity)
                        else:
                            nc.vector.tensor_copy(x1sb[:, md, :, :], psx1[:])

                    # ---- phase 5: output projection ----
                    y_t = []
                    stats = wpool.tile([P, 4], f32, tag="stats", name="stats")
                    ps4t = pp4.tile([P, 2, J], f32, tag="p4", name="ps4")
                    for oc in range(2):
                        ps4 = ps4t[:, oc, :]
                        first = True
                        for c16 in range(16):
                            h, md = c16 // 2, c16 % 2
                            nc.tensor.matmul(ps4, wpl[:, c16, oc, :],
                                             x1sb[:, md, h, :],
                                             start=first, stop=False)
                            first = False
                        for dc in range(2):
                            nc.tensor.matmul(ps4, wnl[:, dc, oc, :],
                                             xtl[:, dc, :],
                                             start=False, stop=(dc == 1))
                        yt = wpool.tile([P, J], f32, tag=f"y{oc}",
                                        name=f"y_{oc}")
                        nc.scalar.activation(yt[:], ps4, AF.Identity,
                                             bias=pvec_sb[:, 2 + oc:3 + oc],
                                             accum_out=stats[:, oc:oc + 1])
                        y_t.append(yt)
                        sq = wpool.tile([P, J], f32, tag="sq", bufs=2,
                                        name=f"sq_{oc}")
                        nc.scalar.activation(sq[:], yt[:], AF.Square,
                                             accum_out=stats[:, 2 + oc:3 + oc])

                    # ---- BN all-reduce + affine + selu (baseline tail) ----
                    cc_in = dpool.tile([P, 4], f32, name="cc_in")
                    cc_out = dpool.tile([P, 4], f32, addr_space="Shared",
                                        name="cc_out")
                    nc.sync.dma_start(cc_in[:], stats[:])
                    if with_collective:
                        nc.gpsimd.collective_compute(
                            "AllReduce",
                            ALU.add,
                            replica_groups=[list(range(NCORES))],
                            ins=[cc_in.opt()],
                            outs=[cc_out.opt()],
                        )
                    else:
                        nc.sync.dma_start(cc_out[:], cc_in[:])
                    statg = wpool.tile([P, 4], f32, tag="statg", name="statg")
                    nc.sync.dma_start(statg[:], cc_out[:])

                    NTOT = float(B * T)

                    def wt2(nm):
                        return wpool.tile([P, 2], f32, tag=nm, name=nm)

                    mom = wpool.tile([P, 4], f32, tag="mom", name="mom")
                    nc.vector.tensor_scalar_mul(out=mom[:, 0:2],
                                                in0=statg[:, 0:2],
                                                scalar1=1.0 / NTOT)
                    nc.vector.tensor_scalar(out=mom[:, 2:4],
                                            in0=statg[:, 2:4],
                                            scalar1=1.0 / NTOT,
                                            scalar2=BN_EPS,
                                            op0=ALU.mult, op1=ALU.add)
                    mu = mom[:, 0:2]
                    varp = mom[:, 2:4]
                    musq = wt2("musq")
                    nc.vector.tensor_mul(musq[:], mu, mu)
                    nc.vector.tensor_sub(varp, varp, musq[:])
                    i32 = mybir.dt.int32
                    magic = wpool.tile([P, 2], i32, tag="magic", name="magic")
                    nc.vector.memset(magic[:], 0x5F3759DF)
                    ri = wpool.tile([P, 2], i32, tag="ri", name="ri")
                    nc.vector.tensor_scalar(out=ri[:], in0=varp.bitcast(i32),
                                            scalar1=1, scalar2=None,
                                            op0=ALU.arith_shift_right)
                    nc.vector.tensor_sub(ri[:], magic[:], ri[:])
                    rstd = wt2("rstd")
                    nc.vector.tensor_copy(rstd[:], ri[:].bitcast(f32))
                    ra = wt2("ra")
                    rb = wt2("rb")
                    for _ in range(2):
                        nc.vector.tensor_mul(ra[:], rstd[:], rstd[:])
                        nc.vector.scalar_tensor_tensor(
                            out=rb[:], in0=ra[:], scalar=-0.5, in1=varp,
                            op0=ALU.mult, op1=ALU.mult)
                        nc.vector.tensor_scalar_add(out=rb[:], in0=rb[:],
                                                    scalar1=1.5)
                        nc.vector.tensor_mul(rstd[:], rstd[:], rb[:])
                    scl = wt2("scl")
                    nc.vector.tensor_mul(scl[:], pvec_sb[:, 4:6], rstd[:])
                    tmp = wt2("tmp")
                    nc.vector.tensor_mul(tmp[:], mu, scl[:])
                    shf = wt2("shf")
                    nc.vector.tensor_sub(shf[:], pvec_sb[:, 6:8], tmp[:])

                    z = wpool.tile([P, 2, J], f32, tag="z", name="z")
                    for oc in range(2):
                        nc.vector.tensor_scalar(out=z[:, oc, :],
                                                in0=y_t[oc][:],
                                                scalar1=scl[:, oc:oc + 1],
                                                scalar2=shf[:, oc:oc + 1],
                                                op0=ALU.mult, op1=ALU.add)
                    neg = wpool.tile([P, 2, J], f32, tag="neg", name="neg")
                    nc.vector.tensor_scalar_min(out=neg[:], in0=z[:],
                                                scalar1=0.0)
                    ep = wpool.tile([P, 2, J], f32, tag="ep", name="ep")
                    nc.scalar.activation(ep[:], neg[:], AF.Exp)
                    em = wpool.tile([P, 2, J], f32, tag="em", name="em")
                    nc.vector.tensor_scalar(
                        out=em[:], in0=ep[:],
                        scalar1=SELU_LAM * SELU_ALPHA,
                        scalar2=-SELU_LAM * SELU_ALPHA,
                        op0=ALU.mult, op1=ALU.add)
                    pos = wpool.tile([P, 2, J], f32, tag="pos", name="pos")
                    nc.vector.tensor_scalar_max(out=pos[:], in0=z[:],
                                                scalar1=0.0)
                    outz = wpool.tile([P, 2, J], f32, tag="outz", name="outz")
                    nc.vector.scalar_tensor_tensor(
                        out=outz[:], in0=pos[:], scalar=SELU_LAM, in1=em[:],
                        op0=ALU.mult, op1=ALU.add)
                    nc.sync.dma_start(yout_d.ap().rearrange("c p j -> p c j"),
                                      outz[:])

    nc.compile()
    return nc


def _prep_inputs(x, boundary, att_proj_w, att_proj_b, att_weight,
                 proj_att_w, proj_att_b, proj_no_w, proj_no_b,
                 bn_gamma, bn_beta):
    x = np.ascontiguousarray(np.asarray(x, dtype=np.float32))
    bnd = np.asarray(boundary)
    mask = _message_control_mask_np(bnd)
    W1 = np.asarray(att_proj_w, np.float32)
    W2 = np.asarray(att_weight, np.float32)
    Wp = np.asarray(proj_att_w, np.float32)
    Wn = np.asarray(proj_no_w, np.float32)

    by = (np.asarray(proj_att_b, np.float32)
          + np.asarray(proj_no_b, np.float32))
    b1 = np.asarray(att_proj_b, np.float32)
    g = np.asarray(bn_gamma, np.float32)
    be = np.asarray(bn_beta, np.float32)
    pvec = np.zeros((P, 8), dtype=np.float32)
    for oc in range(2):
        pvec[:, oc] = b1[oc * P:(oc + 1) * P]
        pvec[:, 2 + oc] = by[oc * P:(oc + 1) * P]
        pvec[:, 4 + oc] = g[oc * P:(oc + 1) * P]
        pvec[:, 6 + oc] = be[oc * P:(oc + 1) * P]

    # shared weight layouts
    W1l = np.zeros((P, 2, 2, P), np.float32)
    WNl = np.zeros((P, 2, 2, P), np.float32)
    for dc in range(2):
        for oc in range(2):
            W1l[:, dc, oc, :] = W1[dc * P:(dc + 1) * P, oc * P:(oc + 1) * P]
            WNl[:, dc, oc, :] = Wn[dc * P:(dc + 1) * P, oc * P:(oc + 1) * P]
    W2c = np.zeros((P, 2, H), np.float32)
    for oc in range(2):
        W2c[:, oc, :] = W2[oc * P:(oc + 1) * P, :]
    Wp3 = Wp.reshape(D, H, O)  # (d, h, o)
    WPl = np.zeros((P, 16, 2, P), np.float32)
    for h in range(H):
        for md in range(2):
            c16 = h * 2 + md
            for oc in range(2):
                WPl[:, c16, oc, :] = Wp3[md * P:(md + 1) * P, h,
                                         oc * P:(oc + 1) * P]

    segs = [_seg_of(bnd[bb]) for bb in range(B)]
    # uniform chunk count across cores (SPMD)
    phat = []
    for c in range(NCORES):
        bb, j0 = c // 2, (c % 2) * J
        phat.append(int(segs[bb][j0:j0 + J, 1].sum()))
    nch = max(1, int(np.ceil(max(phat) / P)))
    _CACHE["nch"] = nch
    PAD = nch * P

    in_maps = []
    for c in range(NCORES):
        bb, j0 = c // 2, (c % 2) * J
        xb = x[bb]  # (T, D)
        seg = segs[bb]
        rows = []   # jl per pair
        keys = []   # global k per pair
        zc = np.zeros((P, 1), np.float32)
        for jl in range(J):
            k0, L = seg[j0 + jl]
            rows += [jl] * int(L)
            keys += list(range(int(k0), int(k0 + L)))
            zc[jl, 0] = float(T - L)
        np_pairs = len(rows)
        assert np_pairs <= PAD
        rows = np.asarray(rows + [0] * (PAD - np_pairs), np.int64)
        keys = np.asarray(keys + [0] * (PAD - np_pairs), np.int64)
        valid = np.zeros(PAD, np.float32)
        valid[:np_pairs] = 1.0

        xT = xb.T  # (D, T)
        XJg = np.zeros((P, 2, PAD), np.float32)
        XKg = np.zeros((P, 2, PAD), np.float32)
        Xp = np.zeros((P, nch, 2, P), np.float32)
        for dc in range(2):
            XJg[:, dc, :] = xT[dc * P:(dc + 1) * P, j0 + rows] * valid
            XKg[:, dc, :] = xT[dc * P:(dc + 1) * P, keys] * valid
        xpk = xb[keys] * valid[:, None]  # (PAD, D)
        for ch in range(nch):
            for md in range(2):
                Xp[:, ch, md, :] = xpk[ch * P:(ch + 1) * P,
                                       md * P:(md + 1) * P]
        JSEL = np.zeros((P, nch, P), np.float32)
        JSELT = np.zeros((P, nch, P), np.float32)
        for p in range(np_pairs):
            ch, pp = p // P, p % P
            JSEL[pp, ch, rows[p]] = 1.0
            JSELT[rows[p], ch, pp] = 1.0
        mrow = mask[bb, j0:j0 + J, :]  # (J, T)
        NOTM = np.zeros((P, 2, P), np.float32)
        for kc in range(2):
            NOTM[:, kc, :] = 1.0 - mrow[:, kc * P:(kc + 1) * P].T
        xkl = np.zeros((P, 2, 2, P), np.float32)
        for kc in range(2):
            for md in range(2):
                xkl[:, kc, md, :] = xb[kc * P:(kc + 1) * P,
                                       md * P:(md + 1) * P]
        xtl = np.zeros((P, 2, P), np.float32)
        for dc in range(2):
            xtl[:, dc, :] = xT[dc * P:(dc + 1) * P, j0:j0 + J]

        in_maps.append({
            "xjg": XJg.astype(BF), "xkg": XKg.astype(BF),
            "xp": Xp.astype(BF), "jsel": JSEL.astype(BF),
            "jselt": JSELT.astype(BF), "notm": NOTM.astype(BF),
            "xk": xkl.astype(BF), "xtl": xtl.astype(BF),
            "w1l": W1l.astype(BF), "w2c": W2c.astype(BF),
            "wpl": WPl.astype(BF), "wnl": WNl.astype(BF),
            "zc": zc, "pvec": pvec,
        })
    return in_maps


def kernel(**inputs):
    from concourse.bass_utils import run_bass_kernel_spmd

    in_maps = _prep_inputs(**inputs)
    nch = _CACHE["nch"]
    key = ("nc", nch)
    if key not in _CACHE:
        _CACHE[key] = _build_module(nch=nch)
    nc = _CACHE[key]

    res = run_bass_kernel_spmd(nc, in_maps, core_ids=list(range(NCORES)),
                               **_CACHE.get("run_kwargs", {}))
    _CACHE["last_results"] = res

    out = np.zeros((B, T, O), dtype=np.float32)
    for c in range(NCORES):
        bb, j0 = c // 2, (c % 2) * J
        yc = res.results[c]["yout"]  # (2, P, J)
        out[bb, j0:j0 + J, :] = yc.reshape(O, J).T
    return out


if __name__ == "__main__":
    _build_module(nch=3)
    print("build ok")


# revision 16
# speedup vs baseline: 3.2418x; 1.0539x over previous
"""Trainium2 Bass kernel for MessageControlGraphAttentionLayer (sparse form).

Key insight: mask[j,k]=1 only when j,k lie in the same zero-run of
boundary (plus the diagonal), so only ~0.8% of (j,k) pairs carry a
computed logit. For masked pairs exp(0)=1, so softmax collapses:
  Z[j,h]   = (T - L_j) + sum_{k in seg(j)} exp(l[j,k,h])
  x1[j,:,h]= ( (A - S_seg(j)) + sum_{k in seg(j)} exp(l)*x[k] ) / Z
with A = sum_k x[k,:] and S_seg = sum_{k in seg} x[k,:]. A - S comes
from one matmul with (1 - mask); the kept pairs live on a compact
pair axis p (host-gathered columns XJg/XKg, selector matrices JSEL).

Per core (batch b=c//2, rows j in [128*(c%2), ...+128)):
  PP[d,p] = XJg*XKg (DVE) -> mm1 (4 matmuls, bf16) -> tanh+b1 (ACT)
  mm2: l[p,h] via lhsT=a-chunk, rhs=W2 (N=8) -> exp -> Z via JSEL
  zi = 1/((T-L)+Z) -> zin gather via JSELT -> en = exp*zin
  JSELh[p,j] = en[:,h]*JSEL (per h); x1T[d,j] += Xp^T @ JSELh
  pseudo-chunk: (A-S)^T with diag(zi_h) rhs handles the masked mass.
  mm4: y = Wp^T x1 + Wn^T x + biases; BN stats AllReduce; affine+selu.

All weights/x in bf16 (validated ~4e-3 end-to-end rel err).
"""

import sys

if "/opt/trn_rl_repo" not in sys.path:
    sys.path.insert(0, "/opt/trn_rl_repo")

import numpy as np
import ml_dtypes

B, T, D, O, H = 4, 256, 256, 256, 8
P = 128
NCORES = 8
J = 128  # rows per core
BN_EPS = 1e-5
SELU_LAM = 1.0507009873554805
SELU_ALPHA = 1.6732632423543772

BF = ml_dtypes.bfloat16

_CACHE = {}


def _message_control_mask_np(boundary):
    Bb, Tt = boundary.shape
    s = np.cumsum(boundary.astype(np.int64), axis=1)
    spad = np.concatenate([np.zeros((Bb, 1), np.int64), s], axis=1)
    idx = np.arange(Tt)
    jj, kk = np.meshgrid(idx, idx, indexing="ij")
    rng_sum = spad[:, np.maximum(jj, kk) + 1] - spad[:, np.minimum(jj, kk)]
    mask = rng_sum == 0
    mask = mask | np.eye(Tt, dtype=bool)[None]
    return mask.astype(np.float32)


def _seg_of(brow):
    """Per-row (k0, L): the maximal zero-run containing the row, or the
    singleton (j, 1) for boundary rows (diagonal-only)."""
    seg = np.zeros((T, 2), np.int64)
    i = 0
    while i < T:
        if brow[i] == 0:
            j = i
            while j < T and brow[j] == 0:
                j += 1
            seg[i:j, 0] = i
            seg[i:j, 1] = j - i
            i = j
        else:
            seg[i] = (i, 1)
            i += 1
    return seg


def _build_module(with_collective=True, reps=1, nch=None):
    from concourse import bacc, tile
    import concourse.mybir as mybir
    from concourse.masks import make_identity

    if nch is None:
        nch = _CACHE.get("nch", 3)
    NCH = nch
    PAD = NCH * P

    f32 = mybir.dt.float32
    bf16 = mybir.dt.bfloat16
    AF = mybir.ActivationFunctionType
    ALU = mybir.AluOpType

    nc = bacc.Bacc("TRN2", target_bir_lowering=False, debug=False,
                   num_devices=NCORES)

    # packed inputs: 4 DMAs instead of 18 (HWDGE queue is 625ns/DMA)
    # pz   f32 [P, 9]: pvec (8) + zc (1)
    # early bf16 [P, 2*PAD + 2*PAD + 512 + 16]: xjg, xkg, w1l, w2c
    # mid  bf16 [P, NCH*128*2 + 256 + 512 + NCH*256 + 256 + 512]:
    #            jsel, jselt, notm, xk, xp, xtl, wnl
    # wpl  bf16 [P, 4096]
    NE = 4 * PAD + 512 + 16
    NM = 2 * NCH * P + 256 + 512 + NCH * 2 * P + 256 + 512
    pz_d = nc.dram_tensor("pz", [P, 9], f32, kind="ExternalInput")
    early_d = nc.dram_tensor("early", [P, NE], bf16, kind="ExternalInput")
    mid_d = nc.dram_tensor("mid", [P, NM], bf16, kind="ExternalInput")
    wpl_d = nc.dram_tensor("wpl", [P, 16, 2, P], bf16, kind="ExternalInput")
    yout_d = nc.dram_tensor("yout", [2, P, J], f32, kind="ExternalOutput")

    with tile.TileContext(nc) as tc:
        with (
            tc.tile_pool(name="const", bufs=1) as cpool,
            tc.tile_pool(name="dram", bufs=1, space="DRAM") as dpool,
        ):
            # Force the ACT table (Tanh/Exp/Identity/Square set) to load
            # before the big const DMAs queue up.
            warm = cpool.tile([P, 1], f32)
            nc.gpsimd.memset(warm[:], 0.0)
            nc.scalar.activation(warm[:], warm[:], AF.Tanh)

            pz_sb = cpool.tile([P, 9], f32)
            nc.sync.dma_start(pz_sb[:], pz_d[:])
            pvec_sb = pz_sb[:, 0:8]
            zc_sb = pz_sb[:, 8:9]
            early = cpool.tile([P, NE], bf16)
            nc.sync.dma_start(early[:], early_d[:])
            o_ = 0
            xjg = early[:, o_:o_ + 2 * PAD].rearrange("p (a b) -> p a b", a=2)
            o_ += 2 * PAD
            xkg = early[:, o_:o_ + 2 * PAD].rearrange("p (a b) -> p a b", a=2)
            o_ += 2 * PAD
            w1_sb = early[:, o_:o_ + 512].rearrange(
                "p (a b c) -> p a b c", a=2, b=2)
            o_ += 512
            w2_sb = early[:, o_:o_ + 16].rearrange("p (a b) -> p a b", a=2)
            mid = cpool.tile([P, NM], bf16)
            nc.sync.dma_start(mid[:], mid_d[:])
            o_ = 0
            jsel = mid[:, o_:o_ + NCH * P].rearrange(
                "p (a b) -> p a b", a=NCH)
            o_ += NCH * P
            jselt = mid[:, o_:o_ + NCH * P].rearrange(
                "p (a b) -> p a b", a=NCH)
            o_ += NCH * P
            notm = mid[:, o_:o_ + 256].rearrange("p (a b) -> p a b", a=2)
            o_ += 256
            xk_sb = mid[:, o_:o_ + 512].rearrange(
                "p (a b c) -> p a b c", a=2, b=2)
            o_ += 512
            xp_sb = mid[:, o_:o_ + NCH * 256].rearrange(
                "p (a b c) -> p a b c", a=NCH, b=2)
            o_ += NCH * 256
            xtl = mid[:, o_:o_ + 256].rearrange("p (a b) -> p a b", a=2)
            o_ += 256
            wnl = mid[:, o_:o_ + 512].rearrange(
                "p (a b c) -> p a b c", a=2, b=2)
            wpl = cpool.tile([P, 16, 2, P], bf16)
            nc.sync.dma_start(wpl[:], wpl_d[:])
            identf = cpool.tile([P, P], f32)
            make_identity(nc, identf[:])
            identr = cpool.tile([P, P], bf16)
            nc.vector.tensor_copy(identr[:], identf[:])

            with (
                tc.tile_pool(name="work", bufs=1) as wpool,
                tc.tile_pool(name="pp1", bufs=1, space="PSUM") as pp1,
                tc.tile_pool(name="pps", bufs=1, space="PSUM") as pps,
                tc.tile_pool(name="ppx", bufs=2, space="PSUM") as ppx,
                tc.tile_pool(name="pp4", bufs=1, space="PSUM") as pp4,
            ):
                for _rep in range(reps):
                    # ---- phase 1: pair products -> mm1 -> tanh ----
                    ppt = wpool.tile([P, 2, PAD], bf16, tag="pp", name="ppt")
                    for dc in range(2):
                        nc.vector.tensor_mul(ppt[:, dc, :], xjg[:, dc, :],
                                             xkg[:, dc, :])
                    ps1 = [pp1.tile([P, PAD], f32, tag=f"p1_{oc}",
                                    name=f"ps1_{oc}") for oc in range(2)]
                    for oc in range(2):
                        for dc in range(2):
                            nc.tensor.matmul(ps1[oc][:],
                                             w1_sb[:, dc, oc, :],
                                             ppt[:, dc, :],
                                             start=(dc == 0), stop=(dc == 1))
                    a_t = wpool.tile([P, 2, PAD], bf16, tag="a", name="a_t")
                    for oc in range(2):
                        nc.scalar.activation(a_t[:, oc, :], ps1[oc][:],
                                             AF.Tanh,
                                             bias=pvec_sb[:, oc:oc + 1])

                    # ---- phase 2: mm2 -> exp per chunk; Z accumulation ----
                    # one shared 2KB psum bank for the tiny psums:
                    # [0:8]=psz, [8:32]=pszn, [32:56]=psl,
                    # [64:192]=transpose scratch (bitcast bf16), [256:512]=AS
                    small = pps.tile([P, 512], f32, tag="small", name="small")
                    psz = small[:, 0:H]
                    expp = wpool.tile([P, NCH, H], bf16, tag="expp",
                                      name="expp")
                    for ch in range(NCH):
                        psl = small[:, 32 + H * ch:32 + H * ch + H]
                        for oc in range(2):
                            nc.tensor.matmul(psl,
                                             a_t[:, oc, ch * P:(ch + 1) * P],
                                             w2_sb[:, oc, :],
                                             start=(oc == 0), stop=(oc == 1))
                        nc.scalar.activation(expp[:, ch, :], psl, AF.Exp)
                    for ch in range(NCH):
                        nc.tensor.matmul(psz, jsel[:, ch, :],
                                         expp[:, ch, :],
                                         start=(ch == 0), stop=(ch == NCH - 1))
                    zs = wpool.tile([P, H], f32, tag="zs", name="zs")
                    nc.vector.tensor_scalar_add(out=zs[:], in0=psz,
                                                scalar1=zc_sb)
                    zi = wpool.tile([P, H], f32, tag="zi", name="zi")
                    nc.vector.reciprocal(zi[:], zs[:])
                    zib = wpool.tile([P, H], bf16, tag="zib", name="zib")
                    nc.vector.tensor_copy(zib[:], zi[:])

                    # ---- A - S side chain (overlaps with above) ----
                    psas = small[:, 256:512].rearrange("p (a b) -> p a b", a=2)
                    for md in range(2):
                        for kc in range(2):
                            nc.tensor.matmul(psas[:, md, :],
                                             xk_sb[:, kc, md, :],
                                             notm[:, kc, :],
                                             start=(kc == 0), stop=(kc == 1))
                    assb = wpool.tile([P, 2, P], bf16, tag="assb", name="assb")
                    nc.scalar.activation(assb[:], psas, AF.Identity)
                    ast = wpool.tile([P, 2, P], bf16, tag="ast", name="ast")
                    pstt = small[:, 64:192].bitcast(bf16).rearrange(
                        "p (a b) -> p a b", a=2)
                    for md in range(2):
                        nc.tensor.transpose(pstt[:, md, :], assb[:, md, :],
                                            identr[:])
                        nc.vector.tensor_copy(ast[:, md, :], pstt[:, md, :])

                    # ---- phase 3: zin gather, en, JSELh builds ----
                    # jh[p,(h,j)] = exp(l)[p,h]*zi[row(p),h]*JSEL[p,j]; the
                    # pseudo chunk diag(zi_h) carries the masked-mass term.
                    jh = wpool.tile([P, NCH, H, P], bf16, tag="jh", name="jh")
                    for ch in range(NCH):
                        pszn = small[:, H + H * ch:H + H * ch + H]
                        nc.tensor.matmul(pszn, jselt[:, ch, :], zib[:],
                                         start=True, stop=True)
                        en = wpool.tile([P, H], f32, tag="en", bufs=3,
                                        name=f"en_{ch}")
                        nc.vector.tensor_mul(en[:], expp[:, ch, :], pszn)
                        for h in range(H):
                            eng = nc.gpsimd if h >= 6 else nc.vector
                            eng.tensor_scalar_mul(out=jh[:, ch, h, :],
                                                  in0=jsel[:, ch, :],
                                                  scalar1=en[:, h:h + 1])
                    jhps = wpool.tile([P, H, P], bf16, tag="jhps", name="jhps")
                    for h in range(H):
                        eng = nc.gpsimd if h >= 6 else nc.vector
                        eng.tensor_scalar_mul(out=jhps[:, h, :],
                                              in0=identr[:],
                                              scalar1=zi[:, h:h + 1])

                    # ---- phase 4: x1T accumulation (2 md waves, h fused) ----
                    x1sb = wpool.tile([P, 2, H, P], bf16, tag="x1sb",
                                      name="x1sb")
                    for md in range(2):
                        psx1 = ppx.tile([P, H, P], f32, tag="psx1",
                                        name=f"psx1_{md}")
                        for hh in range(2):
                            hs = slice(hh * 4, hh * 4 + 4)
                            out = psx1[:, hs, :].rearrange("p a b -> p (a b)")
                            for ch in range(NCH):
                                nc.tensor.matmul(
                                    out, xp_sb[:, ch, md, :],
                                    jh[:, ch, hs, :].rearrange(
                                        "p a b -> p (a b)"),
                                    start=(ch == 0), stop=False)
                            nc.tensor.matmul(
                                out, ast[:, md, :],
                                jhps[:, hs, :].rearrange("p a b -> p (a b)"),
                                start=False, stop=True)
                        if md == 0:
                            nc.scalar.activation(x1sb[:, md, :, :], psx1[:],
                                                 AF.Identity)
                        else:
                            nc.vector.tensor_copy(x1sb[:, md, :, :], psx1[:])

                    # ---- phase 5: output projection ----
                    y_t = []
                    stats = wpool.tile([P, 4], f32, tag="stats", name="stats")
                    ps4t = pp4.tile([P, 2, J], f32, tag="p4", name="ps4")
                    for oc in range(2):
                        ps4 = ps4t[:, oc, :]
                        first = True
                        for c16 in range(16):
                            h, md = c16 // 2, c16 % 2
                            nc.tensor.matmul(ps4, wpl[:, c16, oc, :],
                                             x1sb[:, md, h, :],
                                             start=first, stop=False)
                            first = False
                        for dc in range(2):
                            nc.tensor.matmul(ps4, wnl[:, dc, oc, :],
                                             xtl[:, dc, :],
                                             start=False, stop=(dc == 1))
                    for oc in range(2):
                        yt = wpool.tile([P, J], f32, tag=f"y{oc}",
                                        name=f"y_{oc}")
                        nc.scalar.activation(yt[:], ps4t[:, oc, :],
                                             AF.Identity,
                                             bias=pvec_sb[:, 2 + oc:3 + oc],
                                             accum_out=stats[:, oc:oc + 1])
                        y_t.append(yt)
                        sq = wpool.tile([P, J], f32, tag="sq", bufs=2,
                                        name=f"sq_{oc}")
                        nc.scalar.activation(sq[:], yt[:], AF.Square,
                                             accum_out=stats[:, 2 + oc:3 + oc])

                    # ---- BN all-reduce + affine + selu (baseline tail) ----
                    cc_in = dpool.tile([P, 4], f32, name="cc_in")
                    cc_out = dpool.tile([P, 4], f32, addr_space="Shared",
                                        name="cc_out")
                    nc.sync.dma_start(cc_in[:], stats[:])
                    if with_collective:
                        nc.gpsimd.collective_compute(
                            "AllReduce",
                            ALU.add,
                            replica_groups=[list(range(NCORES))],
                            ins=[cc_in.opt()],
                            outs=[cc_out.opt()],
                        )
                    else:
                        nc.sync.dma_start(cc_out[:], cc_in[:])
                    statg = wpool.tile([P, 4], f32, tag="statg", name="statg")
                    nc.sync.dma_start(statg[:], cc_out[:])

                    NTOT = float(B * T)

                    def wt2(nm):
                        return wpool.tile([P, 2], f32, tag=nm, name=nm)

                    mom = wpool.tile([P, 4], f32, tag="mom", name="mom")
                    nc.vector.tensor_scalar_mul(out=mom[:, 0:2],
                                                in0=statg[:, 0:2],
                                                scalar1=1.0 / NTOT)
                    nc.vector.tensor_scalar(out=mom[:, 2:4],
                                            in0=statg[:, 2:4],
                                            scalar1=1.0 / NTOT,
                                            scalar2=BN_EPS,
                                            op0=ALU.mult, op1=ALU.add)
                    mu = mom[:, 0:2]
                    varp = mom[:, 2:4]
                    musq = wt2("musq")
                    nc.vector.tensor_mul(musq[:], mu, mu)
                    nc.vector.tensor_sub(varp, varp, musq[:])
                    i32 = mybir.dt.int32
                    magic = wpool.tile([P, 2], i32, tag="magic", name="magic")
                    nc.vector.memset(magic[:], 0x5F3759DF)
                    ri = wpool.tile([P, 2], i32, tag="ri", name="ri")
                    nc.vector.tensor_scalar(out=ri[:], in0=varp.bitcast(i32),
                                            scalar1=1, scalar2=None,
                                            op0=ALU.arith_shift_right)
                    nc.vector.tensor_sub(ri[:], magic[:], ri[:])
                    rstd = wt2("rstd")
                    nc.vector.tensor_copy(rstd[:], ri[:].bitcast(f32))
                    ra = wt2("ra")
                    rb = wt2("rb")
                    for _ in range(2):
                        nc.vector.tensor_mul(ra[:], rstd[:], rstd[:])
                        nc.vector.scalar_tensor_tensor(
                            out=rb[:], in0=ra[:], scalar=-0.5, in1=varp,
                            op0=ALU.mult, op1=ALU.mult)
                        nc.vector.tensor_scalar_add(out=rb[:], in0=rb[:],
                                                    scalar1=1.5)
                        nc.vector.tensor_mul(rstd[:], rstd[:], rb[:])
                    scl = wt2("scl")
                    nc.vector.tensor_mul(scl[:], pvec_sb[:, 4:6], rstd[:])
                    tmp = wt2("tmp")
                    nc.vector.tensor_mul(tmp[:], mu, scl[:])
                    shf = wt2("shf")
                    nc.vector.tensor_sub(shf[:], pvec_sb[:, 6:8], tmp[:])

                    z = wpool.tile([P, 2, J], f32, tag="z", name="z")
                    for oc in range(2):
                        nc.vector.tensor_scalar(out=z[:, oc, :],
                                                in0=y_t[oc][:],
                                                scalar1=scl[:, oc:oc + 1],
                                                scalar2=shf[:, oc:oc + 1],
                                                op0=ALU.mult, op1=ALU.add)
                    neg = wpool.tile([P, 2, J], f32, tag="neg", name="neg")
                    nc.vector.tensor_scalar_min(out=neg[:], in0=z[:],
                                                scalar1=0.0)
                    ep = wpool.tile([P, 2, J], f32, tag="ep", name="ep")
                    nc.scalar.activation(ep[:], neg[:], AF.Exp)
                    em = wpool.tile([P, 2, J], f32, tag="em", name="em")
                    nc.vector.tensor_scalar(
                        out=em[:], in0=ep[:],
                        scalar1=SELU_LAM * SELU_ALPHA,
                        scalar2=-SELU_LAM * SELU_ALPHA,
                        op0=ALU.mult, op1=ALU.add)
                    pos = wpool.tile([P, 2, J], f32, tag="pos", name="pos")
                    nc.vector.tensor_scalar_max(out=pos[:], in0=z[:],
                                                scalar1=0.0)
                    outz = wpool.tile([P, 2, J], f32, tag="outz", name="outz")
                    nc.vector.scalar_tensor_tensor(
                        out=outz[:], in0=pos[:], scalar=SELU_LAM, in1=em[:],
                        op0=ALU.mult, op1=ALU.add)
                    nc.sync.dma_start(yout_d.ap().rearrange("c p j -> p c j"),
                                      outz[:])

    nc.compile()
    return nc


def _prep_inputs(x, boundary, att_proj_w, att_proj_b, att_weight,
                 proj_att_w, proj_att_b, proj_no_w, proj_no_b,
                 bn_gamma, bn_beta):
    x = np.ascontiguousarray(np.asarray(x, dtype=np.float32))
    bnd = np.asarray(boundary)
    mask = _message_control_mask_np(bnd)
    W1 = np.asarray(att_proj_w, np.float32)
    W2 = np.asarray(att_weight, np.float32)
    Wp = np.asarray(proj_att_w, np.float32)
    Wn = np.asarray(proj_no_w, np.float32)

    by = (np.asarray(proj_att_b, np.float32)
          + np.asarray(proj_no_b, np.float32))
    b1 = np.asarray(att_proj_b, np.float32)
    g = np.asarray(bn_gamma, np.float32)
    be = np.asarray(bn_beta, np.float32)
    pvec = np.zeros((P, 8), dtype=np.float32)
    for oc in range(2):
        pvec[:, oc] = b1[oc * P:(oc + 1) * P]
        pvec[:, 2 + oc] = by[oc * P:(oc + 1) * P]
        pvec[:, 4 + oc] = g[oc * P:(oc + 1) * P]
        pvec[:, 6 + oc] = be[oc * P:(oc + 1) * P]

    # shared weight layouts
    W1l = np.zeros((P, 2, 2, P), np.float32)
    WNl = np.zeros((P, 2, 2, P), np.float32)
    for dc in range(2):
        for oc in range(2):
            W1l[:, dc, oc, :] = W1[dc * P:(dc + 1) * P, oc * P:(oc + 1) * P]
            WNl[:, dc, oc, :] = Wn[dc * P:(dc + 1) * P, oc * P:(oc + 1) * P]
    W2c = np.zeros((P, 2, H), np.float32)
    for oc in range(2):
        W2c[:, oc, :] = W2[oc * P:(oc + 1) * P, :]
    Wp3 = Wp.reshape(D, H, O)  # (d, h, o)
    WPl = np.zeros((P, 16, 2, P), np.float32)
    for h in range(H):
        for md in range(2):
            c16 = h * 2 + md
            for oc in range(2):
                WPl[:, c16, oc, :] = Wp3[md * P:(md + 1) * P, h,
                                         oc * P:(oc + 1) * P]

    segs = [_seg_of(bnd[bb]) for bb in range(B)]
    # uniform chunk count across cores (SPMD)
    phat = []
    for c in range(NCORES):
        bb, j0 = c // 2, (c % 2) * J
        phat.append(int(segs[bb][j0:j0 + J, 1].sum()))
    nch = max(1, int(np.ceil(max(phat) / P)))
    _CACHE["nch"] = nch
    PAD = nch * P

    in_maps = []
    for c in range(NCORES):
        bb, j0 = c // 2, (c % 2) * J
        xb = x[bb]  # (T, D)
        seg = segs[bb]
        rows = []   # jl per pair
        keys = []   # global k per pair
        zc = np.zeros((P, 1), np.float32)
        for jl in range(J):
            k0, L = seg[j0 + jl]
            rows += [jl] * int(L)
            keys += list(range(int(k0), int(k0 + L)))
            zc[jl, 0] = float(T - L)
        np_pairs = len(rows)
        assert np_pairs <= PAD
        rows = np.asarray(rows + [0] * (PAD - np_pairs), np.int64)
        keys = np.asarray(keys + [0] * (PAD - np_pairs), np.int64)
        valid = np.zeros(PAD, np.float32)
        valid[:np_pairs] = 1.0

        xT = xb.T  # (D, T)
        XJg = np.zeros((P, 2, PAD), np.float32)
        XKg = np.zeros((P, 2, PAD), np.float32)
        Xp = np.zeros((P, nch, 2, P), np.float32)
        for dc in range(2):
            XJg[:, dc, :] = xT[dc * P:(dc + 1) * P, j0 + rows] * valid
            XKg[:, dc, :] = xT[dc * P:(dc + 1) * P, keys] * valid
        xpk = xb[keys] * valid[:, None]  # (PAD, D)
        for ch in range(nch):
            for md in range(2):
                Xp[:, ch, md, :] = xpk[ch * P:(ch + 1) * P,
                                       md * P:(md + 1) * P]
        JSEL = np.zeros((P, nch, P), np.float32)
        JSELT = np.zeros((P, nch, P), np.float32)
        for p in range(np_pairs):
            ch, pp = p // P, p % P
            JSEL[pp, ch, rows[p]] = 1.0
            JSELT[rows[p], ch, pp] = 1.0
        mrow = mask[bb, j0:j0 + J, :]  # (J, T)
        NOTM = np.zeros((P, 2, P), np.float32)
        for kc in range(2):
            NOTM[:, kc, :] = 1.0 - mrow[:, kc * P:(kc + 1) * P].T
        xkl = np.zeros((P, 2, 2, P), np.float32)
        for kc in range(2):
            for md in range(2):
                xkl[:, kc, md, :] = xb[kc * P:(kc + 1) * P,
                                       md * P:(md + 1) * P]
        xtl = np.zeros((P, 2, P), np.float32)
        for dc in range(2):
            xtl[:, dc, :] = xT[dc * P:(dc + 1) * P, j0:j0 + J]

        pzv = np.concatenate([pvec, zc], axis=1)  # (P, 9)
        early = np.concatenate([
            XJg.reshape(P, -1), XKg.reshape(P, -1),
            W1l.reshape(P, -1), W2c.reshape(P, -1)], axis=1)
        mid = np.concatenate([
            JSEL.reshape(P, -1), JSELT.reshape(P, -1),
            NOTM.reshape(P, -1), xkl.reshape(P, -1),
            Xp.reshape(P, -1), xtl.reshape(P, -1),
            WNl.reshape(P, -1)], axis=1)
        in_maps.append({
            "pz": pzv.astype(np.float32),
            "early": early.astype(BF),
            "mid": mid.astype(BF),
            "wpl": WPl.astype(BF),
        })
    return in_maps


def kernel(**inputs):
    from concourse.bass_utils import run_bass_kernel_spmd

    in_maps = _prep_inputs(**inputs)
    nch = _CACHE["nch"]
    key = ("nc", nch)
    if key not in _CACHE:
        _CACHE[key] = _build_module(nch=nch)
    nc = _CACHE[key]

    res = run_bass_kernel_spmd(nc, in_maps, core_ids=list(range(NCORES)),
                               **_CACHE.get("run_kwargs", {}))
    _CACHE["last_results"] = res

    out = np.zeros((B, T, O), dtype=np.float32)
    for c in range(NCORES):
        bb, j0 = c // 2, (c % 2) * J
        yc = res.results[c]["yout"]  # (2, P, J)
        out[bb, j0:j0 + J, :] = yc.reshape(O, J).T
    return out


if __name__ == "__main__":
    _build_module(nch=3)
    print("build ok")
